# revision 9
# baseline (speedup 1.0000x reference)
"""Sliding-window GQA attention (RoPE + tanh soft-cap) on 8 Trainium2 cores.

Sharding: core c = 4*b + hh with b = batch, hh = head-quarter. Each core
handles batch b, q-heads [4*hh, 4*hh+4), kv-heads [2*hh, 2*hh+2) — one
head-group of (4 q-heads, 2 kv-heads); the host sums the 4 partials per batch.

Phases per core:
  A1: q^T = (q_w^T @ x^T), RoPE              -> SBUF qT_sb [128, 8, 2048] f16
  A2: k^T = (k_w^T @ x^T), RoPE              -> SBUF [512, 2048] f16
  A3: v   = (x @ v_w)                        -> SBUF [2048, 512] bf16
  B1: transposed-band attention              -> enc_sb rows [0, 8)
  B2: out = enc^T.T @ o_w over 8 row-tiles   -> DRAM [2048, 3584] partial.

Ring assignment: sync = qw + x stream + output stage; scalar = tables,
sum-row spill, ow loads; gpsimd = kw/vw prefetch (overlaps A1) + sum-row
broadcast. The per-head normalization runs on GpSimd so the slow
DRAM-broadcast round trip never blocks the Vector FIFO. B1 interleaves the
two q-heads of each kv head so one head's softcap/exp latency hides under
the other head's matmuls.
"""

import numpy as np

B, T, D, N, KH, H = 2, 2048, 3584, 16, 8, 256
WINDOW = 1024
SOFT_CAP = 50.0
SCALAR = 0.0625
BASE = 10000.0
NEG = -30000.0  # tanh-domain mask (fits fp16; exp(50*NEG) underflows to 0)

P = 128
NH = 4         # q heads per core
NKH = 2        # kv heads per core
KT = D // P    # 28 contraction tiles
NA = NH * (H // P)   # 8 q^T row-tiles per core
KA = NKH * (H // P)  # 4 k^T row-tiles per core
TB = T // P    # 16 query blocks
TW = 256       # query-pair width for the transposed-attention phase
NCORES = 8

_PROG_CACHE = {}


def _build_program():
    import concourse.bacc as bacc
    import concourse.tile as tile
    import concourse.mybir as mybir

    F32 = mybir.dt.float32
    F16 = mybir.dt.float16
    BF16 = mybir.dt.bfloat16
    Tanh = mybir.ActivationFunctionType.Tanh
    Exp = mybir.ActivationFunctionType.Exp

    nc = bacc.Bacc("TRN2", target_bir_lowering=False, debug=False,
                   num_devices=NCORES)

    xT = nc.dram_tensor("xT", [D, T], F16, kind="ExternalInput")
    qw = nc.dram_tensor("qw", [D, NH * H], F16, kind="ExternalInput")
    kw = nc.dram_tensor("kw", [D, NKH * H], F16, kind="ExternalInput")
    vw = nc.dram_tensor("vw", [D, NKH * H], F16, kind="ExternalInput")
    ow = nc.dram_tensor("ow", [NA * P, D], BF16, kind="ExternalInput")
    cosT = nc.dram_tensor("cosT", [P, T], F16, kind="ExternalInput")
    sinT = nc.dram_tensor("sinT", [P, T], F16, kind="ExternalInput")
    maskt = nc.dram_tensor("maskt", [4 * P, TW], F16, kind="ExternalInput")
    sums_d = nc.dram_tensor("sums_d", [NH, T], BF16, kind="Internal")
    out_p = nc.dram_tensor("out_p", [T, D], F16, kind="ExternalOutput")

    xT_v = xT.ap().rearrange("(o p) t -> p o t", p=P)      # [128, 28, 2048]
    qw_v = qw.ap().rearrange("(o p) h -> p o h", p=P)      # [128, 28, 1024]
    kw_v = kw.ap().rearrange("(o p) h -> p o h", p=P)      # [128, 28, 512]
    vw_v = vw.ap().rearrange("(o p) h -> p o h", p=P)      # [128, 28, 512]
    ow_v = ow.ap().rearrange("(a p) d -> p a d", p=P)      # [128, 8, 3584]

    CH = 256             # t-chunk for projections
    NCH = T // CH        # 8

    def rope_pair(vec, dst_a, dst_b, src_a, src_b, cs, sn, tmp_pool, tag):
        # dst_a = src_a*cos - src_b*sin ; dst_b = src_b*cos + src_a*sin
        t1 = tmp_pool.tile([P, CH], F32, tag=tag)
        t2 = tmp_pool.tile([P, CH], F32, tag=tag)
        vec.tensor_mul(t1, src_a, cs)
        vec.tensor_mul(t2, src_b, sn)
        vec.tensor_sub(dst_a, t1, t2)
        t3 = tmp_pool.tile([P, CH], F32, tag=tag)
        t4 = tmp_pool.tile([P, CH], F32, tag=tag)
        vec.tensor_mul(t3, src_b, cs)
        vec.tensor_mul(t4, src_a, sn)
        vec.tensor_add(dst_b, t3, t4)

    import concourse.bass as bass_mod

    with tile.TileContext(nc) as tc:
        with tc.tile_pool(name="p_tab", bufs=1) as p_tab, \
             tc.tile_pool(name="p_perm", bufs=1) as p_perm:
            # tables ride the scalar HWDGE ring: they must not queue ahead
            # of the weight/x loads on the sync ring.
            cos_sb = p_tab.tile([P, T], F16)
            sin_sb = p_tab.tile([P, T], F16)
            nc.scalar.dma_start(cos_sb[:], cosT.ap())
            nc.scalar.dma_start(sin_sb[:], sinT.ap())
            mk_sb = p_tab.tile([P, 4, TW], F16)
            nc.scalar.dma_start(mk_sb[:], maskt.ap().rearrange(
                "(m p) c -> p m c", p=P))
            bias_c = p_tab.tile([P, 1], F32)
            nc.vector.memset(bias_c[:], -10.0)
            ones_sb = p_tab.tile([P, 1], BF16)
            nc.vector.memset(ones_sb[:], 1.0)

            qT_sb = p_perm.tile([P, NA, T], F16)        # 32 KB/part
            kT_sb = p_perm.tile([P, KA, T], F16)        # 16 KB/part
            v_sb = p_perm.tile([P, TB, NKH * H], BF16)  # 16 KB/part

            with tc.tile_pool(name="p_wkv", bufs=1) as p_wkv:
                # k/v weights prefetch on the gpsimd (SWDGE) ring during A1,
                # into SBUF disjoint from qw so no WAR delay at the A1->A2
                # boundary.
                kw_sb = p_wkv.tile([P, KT, NKH * H], F16, tag="kw")
                vw_sb = p_wkv.tile([P, KT, NKH * H], F16, tag="vw")
                XQ = KT // 4
                for q4 in range(4):
                    ksl = slice(q4 * XQ, (q4 + 1) * XQ)
                    nc.gpsimd.dma_start(kw_sb[:, ksl], kw_v[:, ksl])
                    nc.gpsimd.dma_start(vw_sb[:, ksl], vw_v[:, ksl])

                # ---------------- Phase A1: q^T -> qT_sb ----------------
                with tc.tile_pool(name="p_a1", bufs=1) as pa1, \
                     tc.tile_pool(name="p_a1x", bufs=2) as pa1x, \
                     tc.tile_pool(name="p_a1r", bufs=8) as pa1r, \
                     tc.tile_pool(name="ps_a1", bufs=2, space="PSUM") as ps_a1:
                    qw_sb = pa1.tile([P, KT, NH * H], F16, tag="qw")
                    for q4 in range(4):
                        nc.sync.dma_start(
                            qw_sb[:, q4 * XQ:(q4 + 1) * XQ],
                            qw_v[:, q4 * XQ:(q4 + 1) * XQ])
                    for ch in range(NCH):
                        tsl = slice(ch * CH, (ch + 1) * CH)
                        ps = ps_a1.tile([P, NA, CH], F32, tag="qps")
                        for q4 in range(4):
                            xt = pa1x.tile([P, XQ, CH], F16, tag="xt")
                            nc.sync.dma_start(
                                xt[:], xT_v[:, q4 * XQ:(q4 + 1) * XQ, tsl])
                            for dk in range(XQ):
                                k = q4 * XQ + dk
                                for j in range(NA):
                                    nc.tensor.matmul(
                                        ps[:, j],
                                        qw_sb[:, k, j * P:(j + 1) * P],
                                        xt[:, dk],
                                        start=(k == 0 and j % 2 == 0),
                                        stop=(k == KT - 1),
                                        skip_group_check=True)
                        cs, sn = cos_sb[:, tsl], sin_sb[:, tsl]
                        for pr in range(NA // 2):
                            rope_pair(nc.vector, qT_sb[:, 2 * pr, tsl],
                                      qT_sb[:, 2 * pr + 1, tsl],
                                      ps[:, 2 * pr], ps[:, 2 * pr + 1],
                                      cs, sn, pa1r, "rtmp")

                # ---------- Phase A2+A3: k^T and v, one xT pass ----------
                with tc.tile_pool(name="p_a2x", bufs=2) as pa2x, \
                     tc.tile_pool(name="p_a2r", bufs=8) as pa2r, \
                     tc.tile_pool(name="ps_a2", bufs=2, space="PSUM") as ps_a2, \
                     tc.tile_pool(name="ps_a3", bufs=2,
                                  space="PSUM") as ps_a3:
                    for ch in range(NCH):
                        tsl = slice(ch * CH, (ch + 1) * CH)
                        ps = ps_a2.tile([P, KA, CH], F32, tag="kps")
                        psv = ps_a3.tile([P, CH // P, NKH * H], F32,
                                         tag="vps")
                        for q4 in range(4):
                            xt = pa2x.tile([P, XQ, CH], F16, tag="xt")
                            nc.sync.dma_start(
                                xt[:], xT_v[:, q4 * XQ:(q4 + 1) * XQ, tsl])
                            for dk in range(XQ):
                                k = q4 * XQ + dk
                                for j in range(KA):
                                    nc.tensor.matmul(
                                        ps[:, j],
                                        kw_sb[:, k, j * P:(j + 1) * P],
                                        xt[:, dk],
                                        start=(k == 0 and j % 2 == 0),
                                        stop=(k == KT - 1),
                                        skip_group_check=True)
                                for st in range(CH // P):
                                    nc.tensor.matmul(
                                        psv[:, st],
                                        xt[:, dk, st * P:(st + 1) * P],
                                        vw_sb[:, k], start=(k == 0),
                                        stop=(k == KT - 1))
                        cs, sn = cos_sb[:, tsl], sin_sb[:, tsl]
                        for pr in range(KA // 2):
                            rope_pair(nc.vector, kT_sb[:, 2 * pr, tsl],
                                      kT_sb[:, 2 * pr + 1, tsl],
                                      ps[:, 2 * pr], ps[:, 2 * pr + 1],
                                      cs, sn, pa2r, "rtmp")
                        for st in range(CH // P):
                            nc.vector.tensor_copy(
                                v_sb[:, ch * (CH // P) + st, :], psv[:, st])

            # ---------------- Phase B1: attention -> enc_sb ----------
            with tc.tile_pool(name="p_enc", bufs=1) as p_enc:
              enc_sb = p_enc.tile([P, NA, T], BF16)      # 32 KB/part
              with tc.tile_pool(name="p_b1s", bufs=2) as pb1s, \
                 tc.tile_pool(name="p_b1e", bufs=2) as pb1e, \
                 tc.tile_pool(name="p_sr", bufs=1) as psr, \
                 tc.tile_pool(name="ps_lg", bufs=1, space="PSUM") as ps_lg, \
                 tc.tile_pool(name="ps_sm", bufs=1, space="PSUM") as ps_sm, \
                 tc.tile_pool(name="ps_en", bufs=1, space="PSUM") as ps_en:
                MKJ = {0: 0, 1: 1, 8: 2, 9: 3}

                for kh in range(NKH):
                    srow = [psr.tile([1, T], F32, tag=f"srh{nl}",
                                      name=f"srow{nl}")
                            for nl in range(2)]
                    for pr in range(T // TW):
                        t0p = pr * TW
                        js = max(0, 8 - 2 * pr)
                        jgroups = []
                        j = js
                        while j < 10:
                            w = min(4, 10 - j)
                            jgroups.append((j, w))
                            j += w
                        exps = [pb1e.tile([P, 10, TW], BF16, tag=f"ex{nl}",
                                           name=f"exps{nl}")
                                for nl in range(2)]
                        smp = [ps_sm.tile([P, TW], F32, tag=f"sm{nl}",
                                         name=f"smp{nl}")
                               for nl in range(2)]
                        encp = [ps_en.tile([P, 2, TW], F32, tag=f"en{nl}",
                                          name=f"encp{nl}")
                               for nl in range(2)]
                        # interleave the two q-heads of this kv head: while
                        # head nl=0 waits on tanh/exp, head nl=1's matmuls
                        # keep the PE busy (and vice versa).
                        for gi, (j0, w) in enumerate(jgroups):
                            for nl in range(2):
                                n = kh * 2 + nl
                                lgT = ps_lg.tile([P, 4, TW], F32,
                                                 tag=f"lg{nl}")
                                for dj in range(w):
                                    j = j0 + dj
                                    s0 = (2 * pr - 8 + j) * P
                                    for hh in range(2):
                                        nc.tensor.matmul(
                                            lgT[:, dj],
                                            kT_sb[:, kh * 2 + hh, s0:s0 + P],
                                            qT_sb[:, 2 * n + hh,
                                                  t0p:t0p + TW],
                                            start=(hh == 0 and dj % 2 == 0),
                                            stop=(hh == 1),
                                            skip_group_check=True)
                                tT = pb1s.tile([P, 4, TW], F32,
                                               tag=f"tT{nl}")
                                # q_w ships unscaled; SCALAR folds in here
                                # (logits enter only via tanh).
                                nc.scalar.activation(
                                    tT[:, :w], lgT[:, :w], Tanh,
                                    scale=SCALAR / SOFT_CAP)
                                for dj in range(w):
                                    j = j0 + dj
                                    if j in MKJ:
                                        nc.vector.tensor_add(
                                            tT[:, dj], tT[:, dj],
                                            mk_sb[:, MKJ[j]])
                                nc.scalar.activation(
                                    exps[nl][:, j0:j0 + w], tT[:, :w],
                                    bias=bias_c[:], func=Exp,
                                    scale=SOFT_CAP)
                                for dj in range(w):
                                    nc.tensor.matmul(
                                        smp[nl][0:1, :], ones_sb[:],
                                        exps[nl][:, j0 + dj],
                                        start=(gi == 0 and dj == 0),
                                        stop=(j0 + dj == 9),
                                        skip_group_check=True)
                                for dj in range(w):
                                    j = j0 + dj
                                    stg = 2 * pr - 8 + j
                                    for hh in range(2):
                                        nc.tensor.matmul(
                                            encp[nl][:, hh],
                                            v_sb[:, stg,
                                                 kh * H + hh * P:
                                                 kh * H + (hh + 1) * P],
                                            exps[nl][:, j],
                                            start=(gi == 0 and dj == 0
                                                   and hh == 0),
                                            stop=(j == 9),
                                            skip_group_check=True)
                        for nl in range(2):
                            nc.vector.tensor_copy(
                                srow[nl][:, t0p:t0p + TW], smp[nl][0:1, :])
                            # enc row-tile order: a = kh*4 + nl*2 + hh
                            for hh in range(2):
                                a = 4 * kh + 2 * nl + hh
                                nc.vector.tensor_copy(
                                    enc_sb[:, a, t0p:t0p + TW],
                                    encp[nl][:, hh])
                    # normalize this kv head's two q-heads. reciprocal on
                    # the [1,T] row (vector, cheap), spill on the scalar
                    # ring, broadcast+cast and the enc muls on GpSimd so
                    # the DRAM round trip blocks neither Vector nor Scalar.
                    for nl in range(2):
                        n = kh * 2 + nl
                        nc.vector.reciprocal(srow[nl][:], srow[nl][:])
                        srow_bf = psr.tile([1, T], BF16, tag=f"srb{nl}",
                                           name=f"srow_bf{nl}")
                        nc.vector.tensor_copy(srow_bf[:], srow[nl][:])
                        nrow = sums_d.ap()[n:n + 1, :]
                        nc.scalar.dma_start(nrow, srow_bf[:])
                        bcast = bass_mod.AP(
                            tensor=nrow.tensor, offset=nrow.offset,
                            ap=[[0, P]] + [list(d) for d in nrow.ap[1:]])
                        rbc = psr.tile([P, T], BF16, tag=f"rbc{nl}")
                        nc.scalar.dma_start(rbc[:], bcast)
                        for hh in range(2):
                            a = 4 * kh + 2 * nl + hh
                            nc.gpsimd.tensor_mul(
                                enc_sb[:, a], enc_sb[:, a], rbc[:])

              # ------------- Phase B2: output projection ---------------
              with tc.tile_pool(name="p_b2", bufs=2) as pb2, \
                   tc.tile_pool(name="p_b2o", bufs=2) as pb2o, \
                   tc.tile_pool(name="ps_b2", bufs=2, space="PSUM") as ps_b2:
                    out_b = out_p.ap().rearrange("(tb p) d -> p tb d", p=P)
                    for dch in range(D // 512):
                        dsl = slice(dch * 512, (dch + 1) * 512)
                        ow_sb = pb2.tile([P, NA, 512], BF16, tag="ow")
                        nc.scalar.dma_start(ow_sb[:], ow_v[:, :, dsl])
                        stage = pb2o.tile([P, TB, 512], F16, tag="ob")
                        for tb in range(TB):
                            t0 = tb * P
                            po = ps_b2.tile([P, 512], F32, tag="po")
                            for a in range(NA):
                                nc.tensor.matmul(
                                    po[:], enc_sb[:, a, t0:t0 + P],
                                    ow_sb[:, a],
                                    start=(a == 0), stop=(a == NA - 1))
                            nc.scalar.copy(stage[:, tb], po[:])
                        nc.sync.dma_start(out_b[:, :, dsl], stage[:])

    nc.compile()
    return nc


def _get_program():
    if "nc" not in _PROG_CACHE:
        _PROG_CACHE["nc"] = _build_program()
    return _PROG_CACHE["nc"]


def _host_inputs(x, segment_pos, q_w, kv_w, o_w):
    """Build the 8 per-core input dicts. All large operands ship as fp16."""
    BF = np.float16
    xTs = [np.ascontiguousarray(x[b].T).astype(BF) for b in range(B)]
    tabs = []
    for b in range(B):
        pos = segment_pos[b].astype(np.float64)
        inv_ts = BASE ** (-2.0 * np.arange(H // 2, dtype=np.float64) / H)
        ang = inv_ts[:, None] * pos[None, :]          # [128, T]
        tabs.append((np.cos(ang).astype(BF), np.sin(ang).astype(BF)))

    i = np.arange(P)[:, None]
    c = np.arange(TW)[None, :]
    tiles = []
    for j in (0, 1, 8, 9):
        valid = (c >= P * j + i - WINDOW) & (c <= P * j + i - 1)
        tiles.append(np.where(valid, np.float32(0.0), np.float32(NEG)))
    maskt = np.concatenate(tiles, axis=0).astype(BF)

    in_maps = []
    for core in range(NCORES):
        b, hh = divmod(core, 4)
        # q-heads [4*hh, +4), kv-heads [2*hh, +2)
        qws = np.ascontiguousarray(
            q_w[4 * hh:4 * hh + 4].transpose(1, 0, 2).reshape(
                D, NH * H)).astype(BF)
        kws = np.ascontiguousarray(
            kv_w[0, 2 * hh:2 * hh + 2].transpose(1, 0, 2).reshape(
                D, NKH * H)).astype(BF)
        vws = np.ascontiguousarray(
            kv_w[1, 2 * hh:2 * hh + 2].transpose(1, 0, 2).reshape(
                D, NKH * H)).astype(BF)
        # row-tile order a = kh*4 + nl*2 + hh2 matching B1 writes
        ow_tiles = []
        for a in range(NA):
            kh, r = divmod(a, 4)
            nl, hh2 = divmod(r, 2)
            ow_tiles.append(
                o_w[4 * hh + 2 * kh + nl, hh2 * P:(hh2 + 1) * P, :])
        import ml_dtypes
        ows = np.ascontiguousarray(np.concatenate(ow_tiles, axis=0)).astype(
            ml_dtypes.bfloat16)
        in_maps.append({
            "xT": xTs[b], "qw": qws, "kw": kws, "vw": vws, "ow": ows,
            "cosT": tabs[b][0], "sinT": tabs[b][1], "maskt": maskt,
        })
    return in_maps


def kernel(x, segment_pos, attn_mask, q_w, kv_w, o_w):
    from concourse import bass_utils

    x = np.asarray(x, dtype=np.float32)
    q_w = np.asarray(q_w, dtype=np.float32)
    kv_w = np.asarray(kv_w, dtype=np.float32)
    o_w = np.asarray(o_w, dtype=np.float32)
    segment_pos = np.asarray(segment_pos)

    nc = _get_program()
    in_maps = _host_inputs(x, segment_pos, q_w, kv_w, o_w)
    res = bass_utils.run_bass_kernel_spmd(nc, in_maps,
                                          core_ids=list(range(NCORES)))
    out = np.zeros((B, T, D), dtype=np.float32)
    for core in range(NCORES):
        out[core // 4] += res.results[core]["out_p"].astype(np.float32)
    return out


# revision 16
# speedup vs baseline: 1.1359x; 1.1359x over previous
"""Sliding-window GQA attention (RoPE + tanh soft-cap) on 8 Trainium2 cores.

Sharding: core c = 4*b + hh with b = batch, hh = head-quarter. Each core
handles batch b, q-heads [4*hh, 4*hh+4), kv-heads [2*hh, 2*hh+2) — one
head-group of (4 q-heads, 2 kv-heads); the host sums the 4 partials per batch.

Phases per core:
  A1: q^T = (q_w^T @ x^T), RoPE              -> SBUF qT_sb [128, 8, 2048] f16
  A2: k^T = (k_w^T @ x^T), RoPE              -> SBUF [512, 2048] f16
  A3: v   = (x @ v_w)                        -> SBUF [2048, 512] bf16
  B1: transposed-band attention              -> enc_sb rows [0, 8)
  B2: out = enc^T.T @ o_w over 8 row-tiles   -> DRAM [2048, 3584] partial.

Ring assignment: sync = qw + x stream + output stage; scalar = tables,
sum-row spill, ow loads; gpsimd = kw/vw prefetch (overlaps A1) + sum-row
broadcast. The per-head normalization runs on GpSimd so the slow
DRAM-broadcast round trip never blocks the Vector FIFO. B1 interleaves the
two q-heads of each kv head so one head's softcap/exp latency hides under
the other head's matmuls.
"""

import numpy as np

B, T, D, N, KH, H = 2, 2048, 3584, 16, 8, 256
WINDOW = 1024
SOFT_CAP = 50.0
SCALAR = 0.0625
BASE = 10000.0
NEG = -30000.0  # tanh-domain mask (fits fp16; exp(50*NEG) underflows to 0)

P = 128
NH = 4         # q heads per core
NKH = 2        # kv heads per core
KT = D // P    # 28 contraction tiles
NA = NH * (H // P)   # 8 q^T row-tiles per core
KA = NKH * (H // P)  # 4 k^T row-tiles per core
TB = T // P    # 16 query blocks
TW = 256       # query-pair width for the transposed-attention phase
NCORES = 8

_PROG_CACHE = {}


def _build_program():
    import concourse.bacc as bacc
    import concourse.tile as tile
    import concourse.mybir as mybir

    F32 = mybir.dt.float32
    F16 = mybir.dt.float16
    BF16 = mybir.dt.bfloat16
    Tanh = mybir.ActivationFunctionType.Tanh
    Exp = mybir.ActivationFunctionType.Exp

    nc = bacc.Bacc("TRN2", target_bir_lowering=False, debug=False,
                   num_devices=NCORES)

    xT = nc.dram_tensor("xT", [D, T], F16, kind="ExternalInput")
    qw = nc.dram_tensor("qw", [D, NH * H], F16, kind="ExternalInput")
    kw = nc.dram_tensor("kw", [D, NKH * H], F16, kind="ExternalInput")
    vw = nc.dram_tensor("vw", [D, NKH * H], F16, kind="ExternalInput")
    ow = nc.dram_tensor("ow", [NA * P, D], BF16, kind="ExternalInput")
    cosT = nc.dram_tensor("cosT", [P, T], F16, kind="ExternalInput")
    sinT = nc.dram_tensor("sinT", [P, T], F16, kind="ExternalInput")
    maskt = nc.dram_tensor("maskt", [4 * P, TW], F16, kind="ExternalInput")
    out_p = nc.dram_tensor("out_p", [T, D], F16, kind="ExternalOutput")

    xT_v = xT.ap().rearrange("(o p) t -> p o t", p=P)      # [128, 28, 2048]
    qw_v = qw.ap().rearrange("(o p) h -> p o h", p=P)      # [128, 28, 1024]
    kw_v = kw.ap().rearrange("(o p) h -> p o h", p=P)      # [128, 28, 512]
    vw_v = vw.ap().rearrange("(o p) h -> p o h", p=P)      # [128, 28, 512]
    ow_v = ow.ap().rearrange("(a p) d -> p a d", p=P)      # [128, 8, 3584]

    CH = 256             # t-chunk for projections
    NCH = T // CH        # 8

    def rope_pair(vec, dst_a, dst_b, src_a, src_b, cs, sn, tmp_pool, tag):
        # dst_a = src_a*cos - src_b*sin ; dst_b = src_b*cos + src_a*sin
        t1 = tmp_pool.tile([P, CH], F32, tag=tag)
        t2 = tmp_pool.tile([P, CH], F32, tag=tag)
        vec.tensor_mul(t1, src_a, cs)
        vec.tensor_mul(t2, src_b, sn)
        vec.tensor_sub(dst_a, t1, t2)
        t3 = tmp_pool.tile([P, CH], F32, tag=tag)
        t4 = tmp_pool.tile([P, CH], F32, tag=tag)
        vec.tensor_mul(t3, src_b, cs)
        vec.tensor_mul(t4, src_a, sn)
        vec.tensor_add(dst_b, t3, t4)

    import concourse.bass as bass_mod

    with tile.TileContext(nc) as tc:
        with tc.tile_pool(name="p_tab", bufs=1) as p_tab, \
             tc.tile_pool(name="p_perm", bufs=1) as p_perm:
            # tables ride the scalar HWDGE ring: they must not queue ahead
            # of the weight/x loads on the sync ring.
            cos_sb = p_tab.tile([P, T], F16)
            sin_sb = p_tab.tile([P, T], F16)
            nc.scalar.dma_start(cos_sb[:], cosT.ap())
            nc.scalar.dma_start(sin_sb[:], sinT.ap())
            mk_sb = p_tab.tile([P, 4, TW], F16)
            nc.scalar.dma_start(mk_sb[:], maskt.ap().rearrange(
                "(m p) c -> p m c", p=P))
            bias_c = p_tab.tile([P, 1], F32)
            nc.vector.memset(bias_c[:], -10.0)
            # all-ones [128,128] stationary: the key-sum matmul then lands
            # the per-query sums replicated across all 128 partitions, so
            # normalization needs no DRAM broadcast round trip.
            ones_sb = p_tab.tile([P, P], BF16)
            nc.vector.memset(ones_sb[:], 1.0)

            qT_sb = p_perm.tile([P, NA, T], F16)        # 32 KB/part
            kT_sb = p_perm.tile([P, KA, T], F16)        # 16 KB/part
            v_sb = p_perm.tile([P, TB, NKH * H], BF16)  # 16 KB/part

            with tc.tile_pool(name="p_wkv", bufs=1) as p_wkv:
                # k/v weights prefetch during A1 on the scalar ring (queued
                # behind the small tables), into SBUF disjoint from qw so
                # there is no WAR delay at the A1->A2 boundary and no SDMA
                # contention with the sync ring's critical qw/x loads.
                kw_sb = p_wkv.tile([P, KT, NKH * H], F16, tag="kw")
                vw_sb = p_wkv.tile([P, KT, NKH * H], F16, tag="vw")
                XQ = KT // 4
                for q4 in range(4):
                    ksl = slice(q4 * XQ, (q4 + 1) * XQ)
                    nc.scalar.dma_start(kw_sb[:, ksl], kw_v[:, ksl])
                    nc.scalar.dma_start(vw_sb[:, ksl], vw_v[:, ksl])

                # ---------------- Phase A1: q^T -> qT_sb ----------------
                with tc.tile_pool(name="p_a1", bufs=1) as pa1, \
                     tc.tile_pool(name="p_a1x", bufs=2) as pa1x, \
                     tc.tile_pool(name="p_a1r", bufs=8) as pa1r, \
                     tc.tile_pool(name="ps_a1", bufs=2, space="PSUM") as ps_a1:
                    qw_sb = pa1.tile([P, KT, NH * H], F16, tag="qw")
                    for ch in range(NCH):
                        tsl = slice(ch * CH, (ch + 1) * CH)
                        ps = ps_a1.tile([P, NA, CH], F32, tag="qps")
                        for q4 in range(4):
                            if ch == 0:
                                # interleave qw quarters with ch0's x tiles
                                # on the sync FIFO: the first matmuls start
                                # after ~2.3MB instead of ~9MB.
                                nc.sync.dma_start(
                                    qw_sb[:, q4 * XQ:(q4 + 1) * XQ],
                                    qw_v[:, q4 * XQ:(q4 + 1) * XQ])
                            xt = pa1x.tile([P, XQ, CH], F16, tag="xt")
                            nc.sync.dma_start(
                                xt[:], xT_v[:, q4 * XQ:(q4 + 1) * XQ, tsl])
                            for dk in range(XQ):
                                k = q4 * XQ + dk
                                for j in range(NA):
                                    nc.tensor.matmul(
                                        ps[:, j],
                                        qw_sb[:, k, j * P:(j + 1) * P],
                                        xt[:, dk],
                                        start=(k == 0 and j % 2 == 0),
                                        stop=(k == KT - 1),
                                        skip_group_check=True)
                        cs, sn = cos_sb[:, tsl], sin_sb[:, tsl]
                        for pr in range(NA // 2):
                            rope_pair(nc.vector, qT_sb[:, 2 * pr, tsl],
                                      qT_sb[:, 2 * pr + 1, tsl],
                                      ps[:, 2 * pr], ps[:, 2 * pr + 1],
                                      cs, sn, pa1r, "rtmp")

                # ---------- Phase A2+A3: k^T and v, one xT pass ----------
                with tc.tile_pool(name="p_a2x", bufs=2) as pa2x, \
                     tc.tile_pool(name="p_a2r", bufs=8) as pa2r, \
                     tc.tile_pool(name="ps_a2", bufs=2, space="PSUM") as ps_a2, \
                     tc.tile_pool(name="ps_a3", bufs=2,
                                  space="PSUM") as ps_a3:
                    for ch in range(NCH):
                        tsl = slice(ch * CH, (ch + 1) * CH)
                        ps = ps_a2.tile([P, KA, CH], F32, tag="kps")
                        psv = ps_a3.tile([P, CH // P, NKH * H], F32,
                                         tag="vps")
                        for q4 in range(4):
                            xt = pa2x.tile([P, XQ, CH], F16, tag="xt")
                            nc.sync.dma_start(
                                xt[:], xT_v[:, q4 * XQ:(q4 + 1) * XQ, tsl])
                            for dk in range(XQ):
                                k = q4 * XQ + dk
                                for j in range(KA):
                                    nc.tensor.matmul(
                                        ps[:, j],
                                        kw_sb[:, k, j * P:(j + 1) * P],
                                        xt[:, dk],
                                        start=(k == 0 and j % 2 == 0),
                                        stop=(k == KT - 1),
                                        skip_group_check=True)
                                for st in range(CH // P):
                                    nc.tensor.matmul(
                                        psv[:, st],
                                        xt[:, dk, st * P:(st + 1) * P],
                                        vw_sb[:, k], start=(k == 0),
                                        stop=(k == KT - 1))
                        cs, sn = cos_sb[:, tsl], sin_sb[:, tsl]
                        for pr in range(KA // 2):
                            rope_pair(nc.vector, kT_sb[:, 2 * pr, tsl],
                                      kT_sb[:, 2 * pr + 1, tsl],
                                      ps[:, 2 * pr], ps[:, 2 * pr + 1],
                                      cs, sn, pa2r, "rtmp")
                        for st in range(CH // P):
                            nc.vector.tensor_copy(
                                v_sb[:, ch * (CH // P) + st, :], psv[:, st])

            # ---------------- Phase B1: attention -> enc_sb ----------
            with tc.tile_pool(name="p_enc", bufs=1) as p_enc:
              enc_sb = p_enc.tile([P, NA, T], BF16)      # 32 KB/part
              with tc.tile_pool(name="p_b1s", bufs=2) as pb1s, \
                 tc.tile_pool(name="p_b1e", bufs=2) as pb1e, \
                 tc.tile_pool(name="p_sr", bufs=2) as psr, \
                 tc.tile_pool(name="ps_lg", bufs=1, space="PSUM") as ps_lg, \
                 tc.tile_pool(name="ps_sm", bufs=1, space="PSUM") as ps_sm, \
                 tc.tile_pool(name="ps_en", bufs=1, space="PSUM") as ps_en:
                MKJ = {0: 0, 1: 1, 8: 2, 9: 3}

                for kh in range(NKH):
                    for pr in range(T // TW):
                        t0p = pr * TW
                        js = max(0, 8 - 2 * pr)
                        jgroups = []
                        j = js
                        while j < 10:
                            w = min(4, 10 - j)
                            jgroups.append((j, w))
                            j += w
                        exps = [pb1e.tile([P, 10, TW], BF16, tag=f"ex{nl}",
                                           name=f"exps{nl}")
                                for nl in range(2)]
                        smp = [ps_sm.tile([P, TW], F32, tag=f"sm{nl}",
                                         name=f"smp{nl}")
                               for nl in range(2)]
                        encp = [ps_en.tile([P, 2, TW], F32, tag=f"en{nl}",
                                          name=f"encp{nl}")
                               for nl in range(2)]
                        # interleave the two q-heads of this kv head: while
                        # head nl=0 waits on tanh/exp, head nl=1's matmuls
                        # keep the PE busy (and vice versa).
                        for gi, (j0, w) in enumerate(jgroups):
                            for nl in range(2):
                                n = kh * 2 + nl
                                lgT = ps_lg.tile([P, 4, TW], F32,
                                                 tag=f"lg{nl}")
                                for dj in range(w):
                                    j = j0 + dj
                                    s0 = (2 * pr - 8 + j) * P
                                    for hh in range(2):
                                        nc.tensor.matmul(
                                            lgT[:, dj],
                                            kT_sb[:, kh * 2 + hh, s0:s0 + P],
                                            qT_sb[:, 2 * n + hh,
                                                  t0p:t0p + TW],
                                            start=(hh == 0 and dj % 2 == 0),
                                            stop=(hh == 1),
                                            skip_group_check=True)
                                tT = pb1s.tile([P, 4, TW], F32,
                                               tag=f"tT{nl}")
                                # q_w ships unscaled; SCALAR folds in here
                                # (logits enter only via tanh).
                                nc.scalar.activation(
                                    tT[:, :w], lgT[:, :w], Tanh,
                                    scale=SCALAR / SOFT_CAP)
                                for dj in range(w):
                                    j = j0 + dj
                                    if j in MKJ:
                                        nc.vector.tensor_add(
                                            tT[:, dj], tT[:, dj],
                                            mk_sb[:, MKJ[j]])
                                nc.scalar.activation(
                                    exps[nl][:, j0:j0 + w], tT[:, :w],
                                    bias=bias_c[:], func=Exp,
                                    scale=SOFT_CAP)
                                for dj in range(w):
                                    j = j0 + dj
                                    stg = 2 * pr - 8 + j
                                    for hh in range(2):
                                        nc.tensor.matmul(
                                            encp[nl][:, hh],
                                            v_sb[:, stg,
                                                 kh * H + hh * P:
                                                 kh * H + (hh + 1) * P],
                                            exps[nl][:, j],
                                            start=(gi == 0 and dj == 0
                                                   and hh == 0),
                                            stop=(j == 9),
                                            skip_group_check=True)
                        for nl in range(2):
                            # key-sum burst: one 128-col LDW of the all-ones
                            # stationary, then one matmul per key block. The
                            # result rows are identical across partitions —
                            # the broadcast is free.
                            for jj in range(js, 10):
                                nc.tensor.matmul(
                                    smp[nl][:, :], ones_sb[:],
                                    exps[nl][:, jj],
                                    start=(jj == js), stop=(jj == 9),
                                    skip_group_check=True)
                            rbc = psr.tile([P, TW], F32, tag=f"rb{nl}",
                                           name=f"rbc{nl}")
                            nc.vector.reciprocal_approx_fast(
                                rbc[:], smp[nl][:, :])
                            # enc row-tile order: a = kh*4 + nl*2 + hh;
                            # normalization fuses into the PSUM->SBUF copy.
                            for hh in range(2):
                                a = 4 * kh + 2 * nl + hh
                                nc.vector.tensor_mul(
                                    enc_sb[:, a, t0p:t0p + TW],
                                    encp[nl][:, hh], rbc[:])

              # ------------- Phase B2: output projection ---------------
              with tc.tile_pool(name="p_b2", bufs=2) as pb2, \
                   tc.tile_pool(name="p_b2o", bufs=2) as pb2o, \
                   tc.tile_pool(name="ps_b2", bufs=2, space="PSUM") as ps_b2:
                    out_b = out_p.ap().rearrange("(tb p) d -> p tb d", p=P)
                    for dch in range(D // 512):
                        dsl = slice(dch * 512, (dch + 1) * 512)
                        ow_sb = pb2.tile([P, NA, 512], BF16, tag="ow")
                        nc.scalar.dma_start(ow_sb[:], ow_v[:, :, dsl])
                        stage = pb2o.tile([P, TB, 512], F16, tag="ob")
                        for tb in range(TB):
                            t0 = tb * P
                            po = ps_b2.tile([P, 512], F32, tag="po")
                            for a in range(NA):
                                nc.tensor.matmul(
                                    po[:], enc_sb[:, a, t0:t0 + P],
                                    ow_sb[:, a],
                                    start=(a == 0), stop=(a == NA - 1))
                            nc.scalar.copy(stage[:, tb], po[:])
                        nc.sync.dma_start(out_b[:, :, dsl], stage[:])

    nc.compile()
    return nc


def _get_program():
    if "nc" not in _PROG_CACHE:
        _PROG_CACHE["nc"] = _build_program()
    return _PROG_CACHE["nc"]


def _host_inputs(x, segment_pos, q_w, kv_w, o_w):
    """Build the 8 per-core input dicts. All large operands ship as fp16."""
    BF = np.float16
    xTs = [np.ascontiguousarray(x[b].T).astype(BF) for b in range(B)]
    tabs = []
    for b in range(B):
        pos = segment_pos[b].astype(np.float64)
        inv_ts = BASE ** (-2.0 * np.arange(H // 2, dtype=np.float64) / H)
        ang = inv_ts[:, None] * pos[None, :]          # [128, T]
        tabs.append((np.cos(ang).astype(BF), np.sin(ang).astype(BF)))

    i = np.arange(P)[:, None]
    c = np.arange(TW)[None, :]
    tiles = []
    for j in (0, 1, 8, 9):
        valid = (c >= P * j + i - WINDOW) & (c <= P * j + i - 1)
        tiles.append(np.where(valid, np.float32(0.0), np.float32(NEG)))
    maskt = np.concatenate(tiles, axis=0).astype(BF)

    in_maps = []
    for core in range(NCORES):
        b, hh = divmod(core, 4)
        # q-heads [4*hh, +4), kv-heads [2*hh, +2)
        qws = np.ascontiguousarray(
            q_w[4 * hh:4 * hh + 4].transpose(1, 0, 2).reshape(
                D, NH * H)).astype(BF)
        kws = np.ascontiguousarray(
            kv_w[0, 2 * hh:2 * hh + 2].transpose(1, 0, 2).reshape(
                D, NKH * H)).astype(BF)
        vws = np.ascontiguousarray(
            kv_w[1, 2 * hh:2 * hh + 2].transpose(1, 0, 2).reshape(
                D, NKH * H)).astype(BF)
        # row-tile order a = kh*4 + nl*2 + hh2 matching B1 writes
        ow_tiles = []
        for a in range(NA):
            kh, r = divmod(a, 4)
            nl, hh2 = divmod(r, 2)
            ow_tiles.append(
                o_w[4 * hh + 2 * kh + nl, hh2 * P:(hh2 + 1) * P, :])
        import ml_dtypes
        ows = np.ascontiguousarray(np.concatenate(ow_tiles, axis=0)).astype(
            ml_dtypes.bfloat16)
        in_maps.append({
            "xT": xTs[b], "qw": qws, "kw": kws, "vw": vws, "ow": ows,
            "cosT": tabs[b][0], "sinT": tabs[b][1], "maskt": maskt,
        })
    return in_maps


def kernel(x, segment_pos, attn_mask, q_w, kv_w, o_w):
    from concourse import bass_utils

    x = np.asarray(x, dtype=np.float32)
    q_w = np.asarray(q_w, dtype=np.float32)
    kv_w = np.asarray(kv_w, dtype=np.float32)
    o_w = np.asarray(o_w, dtype=np.float32)
    segment_pos = np.asarray(segment_pos)

    nc = _get_program()
    in_maps = _host_inputs(x, segment_pos, q_w, kv_w, o_w)
    res = bass_utils.run_bass_kernel_spmd(nc, in_maps,
                                          core_ids=list(range(NCORES)))
    out = np.zeros((B, T, D), dtype=np.float32)
    for core in range(NCORES):
        out[core // 4] += res.results[core]["out_p"].astype(np.float32)
    return out
